# revision 9
# baseline (speedup 1.0000x reference)
"""Pairwise cosine similarity  O = (Z/|Z_rows|) @ (Y/|Y_rows|).T  on 8 TRN2 cores.

Sharding: Z rows split across 8 cores (data parallel), Y replicated.
Each core computes a [512, 4096] block of the [4096, 4096] output.

Per-core pipeline (fp32 data, fp32r matmuls = full PE rate):
  1. Load Zc [512, 4096] naturally; row sumsq on the scalar engine (Square
     activation with accum_out); PE-transpose (fp32) into an SBUF-resident
     kxm cache [128, 32k, 512m]; the PSUM->SBUF copyback writes float32r,
     satisfying the fp32r rounding rule. Copybacks are batched 4 transposes
     per DVE instruction to amortize per-instruction overhead.
  2. Stream Y in chunks of 256 rows: row sumsq -> 1/|y| both as a
     per-partition column [128, j] and as a broadcast row [128, n_chunk]
     (tiny PE transpose + gpsimd partition_broadcast); PE-transpose each
     [128,128] block (fp32) into the moving-operand tile [128feat, 256rows]
     (fp32r via copyback), then matmul-accumulate over 32 k-tiles into 4
     PSUM banks.
  3. Evict PSUM with per-partition 1/|z| scale and broadcast 1/|y| row
     scale, DMA to the output block.
"""

import os
import sys
import numpy as np

_TRN_REPO = "/opt/trn_rl_repo"
if _TRN_REPO not in sys.path:
    sys.path.insert(0, _TRN_REPO)

import concourse.bacc as bacc
import concourse.mybir as mybir
import concourse.tile as tile
from concourse.bass_utils import run_bass_kernel_spmd
from concourse.masks import make_identity

P = 128
N_CORES = 8
F32 = mybir.dt.float32
F32R = mybir.dt.float32r


def build(bz_core=512, by=4096, feat=4096, n_chunk=256):
    """Build + bacc-compile the SPMD program (same program on every core)."""
    assert bz_core % P == 0 and by % n_chunk == 0 and feat % P == 0
    assert n_chunk % P == 0 and (feat // P) % 4 == 0
    m_sub = bz_core // P          # output row sub-tiles (PSUM banks)
    k_tiles = feat // P           # contraction tiles
    n_chunks = by // n_chunk      # Y row chunks
    j_sub = n_chunk // P          # row sub-tiles per Y chunk

    nc = bacc.Bacc("TRN2", target_bir_lowering=False, debug=False,
                   num_devices=N_CORES)
    z = nc.dram_tensor("z", [bz_core, feat], F32, kind="ExternalInput").ap()
    y = nc.dram_tensor("y", [by, feat], F32, kind="ExternalInput").ap()
    o = nc.dram_tensor("o", [bz_core, by], F32, kind="ExternalOutput").ap()

    with tile.TileContext(nc) as tc:
        with tc.tile_pool(name="const", bufs=1) as const_pool, \
             tc.tile_pool(name="kxm", bufs=1) as kxm_pool, \
             tc.tile_pool(name="nat", bufs=2) as nat_pool, \
             tc.tile_pool(name="small", bufs=2) as small_pool, \
             tc.tile_pool(name="sq", bufs=2) as sq_pool, \
             tc.tile_pool(name="yt", bufs=3) as yt_pool, \
             tc.tile_pool(name="outs", bufs=3) as out_pool, \
             tc.tile_pool(name="pacc", bufs=1, space="PSUM") as pacc_pool, \
             tc.tile_pool(name="ptr", bufs=3, space="PSUM") as ptr_pool:

            ident32 = const_pool.tile([P, P], F32)
            make_identity(nc, ident32)

            def row_rnorm(nat_ap, rdst):
                """rdst[p,0] = 1/|row p| for a [P, feat] natural tile.

                Squares+partial sums on the (otherwise idle) scalar engine.
                """
                parts = small_pool.tile([P, feat // 512], F32, tag="parts")
                for s in range(feat // 512):
                    sq = sq_pool.tile([P, 512], F32, tag="sqscratch")
                    nc.scalar.activation(
                        sq[:], nat_ap[:, s * 512:(s + 1) * 512],
                        mybir.ActivationFunctionType.Square,
                        accum_out=parts[:, s:s + 1])
                ss = small_pool.tile([P, 1], F32, tag="ss")
                nc.vector.reduce_sum(ss[:], parts[:], axis=mybir.AxisListType.X)
                std = small_pool.tile([P, 1], F32, tag="std")
                nc.scalar.sqrt(std[:], ss[:])
                nc.vector.reciprocal(rdst, std[:])

            # ---- Z phase: norms + transpose into kxm cache ----
            zn = nat_pool.tile([P, m_sub, feat], F32, tag="nat")
            nc.sync.dma_start(
                out=zn[:], in_=z.rearrange("(j p) f -> p j f", p=P))
            rz = const_pool.tile([P, m_sub], F32)
            for j in range(m_sub):
                row_rnorm(zn[:, j], rz[:, j:j + 1])
            kxm = kxm_pool.tile([P, k_tiles, bz_core], F32R)
            for j in range(m_sub):
                for k0 in range(0, k_tiles, 4):
                    pt = ptr_pool.tile([P, 512], F32, tag="ptp")
                    for i in range(4):
                        nc.tensor.transpose(
                            pt[:, i * P:(i + 1) * P],
                            zn[:, j, (k0 + i) * P:(k0 + i + 1) * P],
                            ident32[:])
                    nc.vector.tensor_copy(
                        kxm[:, k0:k0 + 4, j * P:(j + 1) * P],
                        pt[:].rearrange("p (i q) -> p i q", i=4))

            # ---- main loop over Y chunks ----
            for c in range(n_chunks):
                ynat = nat_pool.tile([P, j_sub, feat], F32, tag="nat")
                nc.sync.dma_start(
                    out=ynat[:],
                    in_=y[c * n_chunk:(c + 1) * n_chunk, :].rearrange(
                        "(j p) f -> p j f", p=P))
                ry = small_pool.tile([P, j_sub], F32, tag="ry")
                for j in range(j_sub):
                    row_rnorm(ynat[:, j], ry[:, j:j + 1])
                # build broadcast row of 1/|y|: [1, n_chunk] -> [128, n_chunk]
                ryrow = small_pool.tile([P, n_chunk], F32, tag="ryrow")
                for j in range(j_sub):
                    pt = ptr_pool.tile([P, 512], F32, tag="ptp")
                    nc.tensor.transpose(pt[:1, :P], ry[:, j:j + 1], ident32[:])
                    nc.vector.tensor_copy(
                        ryrow[:1, j * P:(j + 1) * P], pt[:1, :P])
                ryb = small_pool.tile([P, n_chunk], F32, tag="ryb")
                nc.gpsimd.partition_broadcast(ryb[:], ryrow[:1, :])

                accs = [pacc_pool.tile([P, n_chunk], F32, tag=f"acc{m}",
                                       name=f"acc{m}")
                        for m in range(m_sub)]
                for k in range(k_tiles):
                    yt = yt_pool.tile([P, n_chunk], F32R, tag="yt")
                    pt = ptr_pool.tile([P, 512], F32, tag="ptp")
                    for j in range(j_sub):
                        nc.tensor.transpose(
                            pt[:, j * P:(j + 1) * P],
                            ynat[:, j, k * P:(k + 1) * P], ident32[:])
                    nc.vector.tensor_copy(yt[:], pt[:, :n_chunk])
                    for m in range(m_sub):
                        nc.tensor.matmul(
                            accs[m][:], kxm[:, k, m * P:(m + 1) * P], yt[:],
                            start=(k == 0), stop=(k == k_tiles - 1))
                for m in range(m_sub):
                    ob = out_pool.tile([P, n_chunk], F32, tag="ob")
                    nc.vector.tensor_scalar_mul(ob[:], accs[m][:],
                                                rz[:, m:m + 1])
                    nc.vector.tensor_mul(ob[:], ob[:], ryb[:])
                    nc.sync.dma_start(
                        out=o[m * P:(m + 1) * P,
                              c * n_chunk:(c + 1) * n_chunk],
                        in_=ob[:])

    nc.compile()
    return nc


_CACHE = {}


def _get_compiled():
    if "nc" not in _CACHE:
        _CACHE["nc"] = build()
    return _CACHE["nc"]


def kernel(Z, Y):
    Z = np.ascontiguousarray(np.asarray(Z, dtype=np.float32))
    Y = np.ascontiguousarray(np.asarray(Y, dtype=np.float32))
    bz = Z.shape[0]
    shard = bz // N_CORES
    nc = _get_compiled()
    in_maps = [{"z": Z[i * shard:(i + 1) * shard], "y": Y}
               for i in range(N_CORES)]
    res = run_bass_kernel_spmd(nc, in_maps, list(range(N_CORES)))
    out = np.concatenate([res.results[i]["o"] for i in range(N_CORES)], axis=0)
    return out
